# revision 35
# baseline (speedup 1.0000x reference)
"""Trainium2 Bass kernel for nn_Lorec (moe_routing LoRA-with-soft-routing).

Computation (per batch b):
  gate_b = softmax(MLP(LayerNorm(ctr[b])))                    [16]
  A_b[i,r] = sum_r' Wa[r*4096+i, r'] gate_b[r']               [4096,16]
  B_b[r,o] = sum_r' Wb[r*4096+o, r'] gate_b[r']               [16,4096]
  out[b] = (x[b] @ A_b) @ B_b * 2.0                           [2048,4096]

Sharding: data-parallel over bs=8 across 8 NeuronCores (one batch per core).
Gating replicated on every core (tiny); each core selects its own batch via a
one-hot `sel` input baked into the packed gating constants.

DMA-optimized: all bulk HBM traffic is bf16 (x in, y out, Wa/Wb), x is
pre-transposed on the host into xt[sb*128+p, c*512+s] = x[sb*512+s, c*128+p]
so mm1 consumes natural tiles with i on partitions (no device transposes).
Per-core HBM traffic ~36 MB (16 x + 16 y + 4 W).

PE-utilization tricks:
  - softmax denominator folded out: gate used UNNORMALIZED (exp only); the
    1/sum^2 factor is broadcast to [128,1] and folded into the PSUM->SBUF
    output copies (out is bilinear in gate).
  - mm1 (M=16): 2-way PE col-tiling -> psxa4 holds xa^T replicated at
    partition offsets 0/32 (c=0 uses a zero-padded full-width lhsT to
    initialize the whole PSUM bank).
  - mm2 (K=16): 2-way PE row-tiling -> t-tiles t,t+1 computed concurrently
    from xaT2/B_sb2 replicas at partition offsets 0/32.
  - PSUM->SBUF output copies rotate over ACT/DVE/GPSIMD.
  - all gating constants arrive in ONE packed [128,1204] f32 DMA.
"""

import sys

sys.path.insert(0, "/opt/trn_rl_repo")

import numpy as np
import ml_dtypes

BF16 = ml_dtypes.bfloat16

BS = 8
SEQ = 2048
IN = 4096
OUT = 4096
R = 16
CTR_OUT = 256
CTR_HID = 60
FD = 16  # FINAL_DIM
LN_EPS = 1e-5
SCALING = 2.0

P = 128
NSB = 4  # s-blocks per core
SBW = 512  # s-block width
NC_I = IN // P  # 32 i-chunks
NOB = OUT // 512  # 8 o-blocks

# packed gating tensor column offsets
CTR0 = 0
W1T0 = 256
W2T0 = 376
B10 = 392
B20 = 393
SEL0 = 394
I16T0 = 402
KRON0 = 530
EPS0 = 562
ONE16 = 563
ONE128 = 564
GAM0 = 692
BET0 = 948
GPC = 1204

_COMPILED = None


def build_program():
    import concourse.bass as bass
    import concourse.mybir as mybir
    from concourse import bacc
    from concourse.masks import make_identity
    from concourse.tile import TileContext

    f32 = mybir.dt.float32
    bf16 = mybir.dt.bfloat16
    AX = mybir.AxisListType.X
    ALU = mybir.AluOpType
    ACTF = mybir.ActivationFunctionType

    nc = bacc.Bacc("TRN2", target_bir_lowering=False, debug=False, num_devices=BS)

    xt_d = nc.dram_tensor("xt", [NSB * P, NC_I * SBW], bf16, kind="ExternalInput").ap()
    gpk_d = nc.dram_tensor("gpk", [P, GPC], f32, kind="ExternalInput").ap()
    wab_d = nc.dram_tensor("wab", [P, 2 * IN + 2 * OUT], bf16, kind="ExternalInput").ap()
    y_d = nc.dram_tensor("y", [2 * NSB * P, 2 * OUT], bf16, kind="ExternalOutput").ap()

    with TileContext(nc) as tc:
        with (
            tc.tile_pool(name="const", bufs=1) as const,
            tc.tile_pool(name="gp", bufs=1) as gp,
            tc.tile_pool(name="wpool", bufs=1) as wpool,
            tc.tile_pool(name="xpool", bufs=3) as xpool,
            tc.tile_pool(name="xapool", bufs=2) as xapool,
            tc.tile_pool(name="opool", bufs=3) as opool,
            tc.tile_pool(name="psxa_pool", bufs=2, space="PSUM") as psxa_pool,
            tc.tile_pool(name="pso_pool", bufs=5, space="PSUM") as pso_pool,
            tc.tile_pool(name="psg_pool", bufs=1, space="PSUM") as psg_pool,
        ):
            # ---- big-stream DMAs, queued on the sync (SP HWDGE) ring ----
            gpk = gp.tile([P, GPC], f32)
            nc.scalar.dma_start(out=gpk[:], in_=gpk_d[:])
            # all adapter weights in ONE 4 MB DMA (large transfers run at
            # ~90% of peak vs ~78% for 1 MB)
            wab = wpool.tile([P, 2 * IN + 2 * OUT], bf16)
            nc.sync.dma_start(out=wab[:], in_=wab_d[:])
            waps = [wab[:, h * IN : (h + 1) * IN] for h in range(2)]
            wbps = [wab[:, 2 * IN + h * OUT : 2 * IN + (h + 1) * OUT] for h in range(2)]
            xsbs = {}
            for sb in range(2):
                xsb = xpool.tile([P, NC_I * SBW], bf16, tag="xsb", name="xsb")
                nc.sync.dma_start(out=xsb[:], in_=xt_d[sb * P : (sb + 1) * P, :])
                xsbs[sb] = xsb

            ident = const.tile([P, P], f32)
            make_identity(nc, ident)

            # slices of the packed gating tile
            ctr = gpk[0:BS, CTR0 : CTR0 + CTR_OUT]
            gam = gpk[0:BS, GAM0 : GAM0 + CTR_OUT]
            bet = gpk[0:BS, BET0 : BET0 + CTR_OUT]
            w1t = gpk[0:P, W1T0 : W1T0 + 2 * CTR_HID]
            w2t = gpk[0:CTR_HID, W2T0 : W2T0 + FD]
            b1 = gpk[0:CTR_HID, B10 : B10 + 1]
            b2 = gpk[0:FD, B20 : B20 + 1]
            sel = gpk[0:FD, SEL0 : SEL0 + BS]
            i16t = gpk[0:FD, I16T0 : I16T0 + P]
            kron = gpk[0:P, KRON0 : KRON0 + 2 * FD]
            eps = gpk[0:BS, EPS0 : EPS0 + 1]
            one16 = gpk[0:FD, ONE16 : ONE16 + 1]
            one128 = gpk[0:1, ONE128 : ONE128 + P]

            # ---- LayerNorm on [8, 256] ----
            mean = gp.tile([BS, 1], f32)
            xc = gp.tile([BS, CTR_OUT], f32)
            sq = gp.tile([BS, CTR_OUT], f32)
            vs = gp.tile([BS, 1], f32)
            std = gp.tile([BS, 1], f32)
            rstd = gp.tile([BS, 1], f32)
            hh = gp.tile([BS, CTR_OUT], f32)
            # ops grouped by engine (DVE runs, then ACT runs) to minimize
            # cross-engine semaphore hops on this serial critical path
            nc.vector.tensor_reduce(mean[:], ctr, axis=AX, op=ALU.add)
            nc.vector.tensor_scalar_mul(mean[:], mean[:], 1.0 / CTR_OUT)
            nc.vector.tensor_scalar_sub(xc[:], ctr, mean[:])
            nc.scalar.activation(sq[:], xc[:], ACTF.Square, accum_out=vs[:])
            nc.scalar.activation(std[:], vs[:], ACTF.Sqrt, bias=eps, scale=1.0 / CTR_OUT)
            nc.vector.reciprocal(rstd[:], std[:])
            nc.vector.tensor_scalar_mul(hh[:], xc[:], rstd[:])
            nc.vector.tensor_mul(hh[:], hh[:], gam)
            nc.vector.tensor_add(hh[:], hh[:], bet)

            # ---- hT [256->2x128, 8] via PE transpose ----
            hT = gp.tile([P, 2 * BS], f32)
            for h in range(2):
                pt = psg_pool.tile([P, BS], f32, tag="psg_small")
                nc.tensor.transpose(pt[:], hh[:, h * P : (h + 1) * P], ident[0:BS, 0:BS])
                nc.scalar.copy(hT[:, h * BS : (h + 1) * BS], pt[:])

            # ---- h1T = relu(W1 @ h + b1) -> [60, 8] ----
            ph1 = psg_pool.tile([CTR_HID, BS], f32, tag="psg_small")
            for h in range(2):
                nc.tensor.matmul(
                    ph1[:], w1t[:, h * CTR_HID : (h + 1) * CTR_HID],
                    hT[:, h * BS : (h + 1) * BS], start=(h == 0), stop=(h == 1),
                )
            h1T = gp.tile([CTR_HID, BS], f32)
            nc.scalar.activation(h1T[:], ph1[:], ACTF.Relu, bias=b1)

            # ---- logitsT = W2 @ h1 + b2 -> [16, 8] ----
            plog = psg_pool.tile([FD, BS], f32, tag="psg_small")
            nc.tensor.matmul(plog[:], w2t, h1T[:], start=True, stop=True)
            logitsT = gp.tile([FD, BS], f32)
            nc.scalar.activation(logitsT[:], plog[:], ACTF.Identity, bias=b2)

            # ---- select own batch, unnormalized gate e = exp(logit_b) ----
            lsel = gp.tile([FD, BS], f32)
            logit_b = gp.tile([FD, 1], f32)
            nc.vector.tensor_mul(lsel[:], logitsT[:], sel)
            nc.vector.tensor_reduce(logit_b[:], lsel[:], axis=AX, op=ALU.add)
            eb = gp.tile([FD, 1], f32)
            nc.scalar.activation(eb[:], logit_b[:], ACTF.Exp)

            # ---- G = I_16 kron e, layout [128, 2*16] bf16 via mask*bcast ----
            pgt = psg_pool.tile([P, 1], f32, tag="psg_small")
            nc.tensor.matmul(pgt[:], i16t, eb[:], start=True, stop=True)
            gtile = gp.tile([P, 1], f32)
            nc.scalar.copy(gtile[:], pgt[:])
            G = gp.tile([P, 2 * FD], bf16)
            nc.vector.tensor_scalar_mul(G[:], kron, gtile[:])
            # Gz: G halves zero-padded to 32 columns each. All down-stream
            # matmuls use 32-wide stationary operands so every PE col/row
            # strip is nominally active (keeps the HAM clock-gate at 8/8).
            Gz = gp.tile([P, 2 * 32], bf16)
            nc.gpsimd.memset(Gz[:], 0.0)
            for h in range(2):
                nc.scalar.copy(Gz[:, h * 32 : h * 32 + FD], G[:, h * FD : (h + 1) * FD])

            # ---- rsq = 1/sum(e)^2 broadcast to [128,1] (off critical path) ----
            psum1 = psg_pool.tile([1, 1], f32, tag="psg_small")
            nc.tensor.matmul(psum1[:], one16, eb[:], start=True, stop=True)
            ssum = gp.tile([1, 1], f32)
            nc.vector.tensor_copy(ssum[:], psum1[:])
            rs = gp.tile([1, 1], f32)
            nc.vector.reciprocal(rs[:], ssum[:])
            rs2 = gp.tile([1, 1], f32)
            nc.vector.tensor_mul(rs2[:], rs[:], rs[:])
            prsq = psg_pool.tile([P, 1], f32, tag="psg_small")
            nc.tensor.matmul(prsq[:], one128, rs2[:], start=True, stop=True)
            rsq = gp.tile([P, 1], f32)
            nc.scalar.copy(rsq[:], prsq[:])

            # ---- A-gen: A_sb[p, c*32+r] = A[c*128+p, r] (bf16, zero-padded to
            # 32-col blocks; padding comes from Gz's zero columns) ----
            A_sb = gp.tile([P, NC_I * 32], bf16)
            for half in range(2):
                psA = psxa_pool.tile([P, 512], f32, tag="psmm", name="psA")
                for cl in range(NC_I // 2):
                    c = half * (NC_I // 2) + cl
                    for h in range(2):
                        nc.tensor.matmul(
                            psA[:, cl * 32 : (cl + 1) * 32],
                            waps[h][:, c * P : (c + 1) * P],
                            Gz[:, h * 32 : (h + 1) * 32],
                            start=(h == 0), stop=(h == 1),
                        )
                nc.scalar.copy(A_sb[:, half * 512 : (half + 1) * 512], psA[:])

            # ---- B-gen: B_sb4 [128, 4096] bf16 = B replicated at 0/32/64/96 ----
            # (emitted after mm1(sb0) in the main loop so the PE doesn't stall
            # on the later-arriving wbp DMAs before starting mm1)
            B_sb4 = gp.tile([P, OUT], bf16)

            def emit_bgen():
                for ob in range(NOB):
                    psB = psxa_pool.tile([P, 512], f32, tag="psmm", name="psB")
                    for h in range(2):
                        for j in range(4):
                            nc.tensor.matmul(
                                psB[32 * j : 32 * j + 32, :],
                                Gz[:, h * 32 : (h + 1) * 32],
                                wbps[h][:, ob * 512 : (ob + 1) * 512],
                                start=(h == 0), stop=(h == 1), skip_group_check=True,
                                tile_position=(0, 32 * j),
                            )
                    if ob % 2 == 0:
                        nc.scalar.copy(B_sb4[:, ob * 512 : (ob + 1) * 512], psB[:])
                    else:
                        nc.vector.tensor_copy(B_sb4[:, ob * 512 : (ob + 1) * 512], psB[:])

            # ---- main loop over s-blocks ----
            # mm2 rounds of s-block sb-1 are interleaved into mm1(sb)'s
            # instruction stream so PSUM->SBUF copies drain in parallel and
            # y DMAs spread through the loop instead of bursting at the end.
            # Half-s-block pipelining: 8 stages of 256 seq positions. Each
            # stage's mm2 is only 4 rounds (4-way: 2 t-tiles x 2 ob-halves),
            # interleaved into the next stage's mm1, so the un-overlapped
            # drain at the end of the kernel is one half-block.
            HB = SBW // 2  # 256

            def emit_mm2_round(pd):
                r = pd["ops"].pop(0)  # 0..3; round r completes obs {2r, 2r+1}
                xa = pd["xaT4"]
                psos = []
                for g in range(4):
                    t = g % 2
                    ob = 2 * r + g // 2
                    pso = pso_pool.tile([P, 512], f32, tag="pso")
                    nc.tensor.matmul(
                        pso[:],
                        xa[32 * g : 32 * g + 32, t * P : (t + 1) * P],
                        B_sb4[32 * g : 32 * g + 32, ob * 512 : (ob + 1) * 512],
                        start=True, stop=True,
                        tile_position=(32 * g, 0),
                    )
                    psos.append(pso)
                for g in range(4):
                    t = g % 2
                    ob = 2 * r + g // 2
                    dst = pd["out"][:, t * OUT + ob * 512 : t * OUT + (ob + 1) * 512]
                    if g % 2 == 0:
                        nc.scalar.activation(dst, psos[g][:], ACTF.Copy, scale=rsq[:])
                    else:
                        nc.vector.tensor_scalar_mul(dst, psos[g][:], rsq[:])
                i = pd["h"]
                if pd["h"] == 2 * NSB - 1:
                    # final stage: release y in 512 KB halves as soon as each
                    # ob-half completes, shrinking the end-of-kernel drain
                    if r == 1 or r == 3:
                        c0 = 0 if r == 1 else 2048
                        for t in range(2):
                            nc.scalar.dma_start(
                                out=y_d[i * P : (i + 1) * P, t * OUT + c0 : t * OUT + c0 + 2048],
                                in_=pd["out"][:, t * OUT + c0 : t * OUT + c0 + 2048],
                            )
                elif not pd["ops"]:
                    nc.scalar.dma_start(
                        out=y_d[i * P : (i + 1) * P, :], in_=pd["out"][:]
                    )

            pend = None
            for h in range(2 * NSB):
                sbq, s0 = h // 2, (h % 2) * HB
                if h % 2 == 0 and sbq + 2 < NSB:
                    nsb = sbq + 2
                    xsb_n = xpool.tile([P, NC_I * SBW], bf16, tag="xsb")
                    nc.sync.dma_start(
                        out=xsb_n[:], in_=xt_d[nsb * P : (nsb + 1) * P, :]
                    )
                    xsbs[nsb] = xsb_n
                xsb = xsbs[sbq]

                # mm1: xa^T replicated at partition offsets 0/32/64/96, each
                # group 32 cols wide (16 real + 16 zero-pad) -> full PE width
                psxa4 = psxa_pool.tile([P, HB], f32, tag="psmm", name="psxa4")
                for c in range(NC_I):
                    for j in range(4):
                        nc.tensor.matmul(
                            psxa4[32 * j : 32 * j + 32, :],
                            A_sb[:, c * 32 : (c + 1) * 32],
                            xsb[:, c * SBW + s0 : c * SBW + s0 + HB],
                            start=(c == 0), stop=(c == NC_I - 1), skip_group_check=True,
                            tile_position=(0, 32 * j),
                        )
                    if c % 8 == 4 and pend is not None and pend["ops"]:
                        emit_mm2_round(pend)
                xaT4 = xapool.tile([P, HB], bf16, tag="xaT")
                nc.vector.tensor_copy(xaT4[:], psxa4[:])
                while pend is not None and pend["ops"]:
                    emit_mm2_round(pend)
                if h == 0:
                    emit_bgen()
                pend = dict(
                    h=h,
                    xaT4=xaT4,
                    out=opool.tile([P, 2 * OUT], bf16, tag="osb", name="outT"),
                    ops=list(range(4)),
                )
            while pend["ops"]:
                emit_mm2_round(pend)

    nc.compile()
    return nc


def host_prep(inputs):
    """Build per-core and shared input arrays from the full problem inputs."""
    x = np.asarray(inputs["x"], np.float32)
    ctr = np.asarray(inputs["ctr_hidden_states"], np.float32)
    gam = np.asarray(inputs["ln_gamma"], np.float32)
    bet = np.asarray(inputs["ln_beta"], np.float32)
    W1 = np.asarray(inputs["W1"], np.float32)
    w1t = np.ascontiguousarray(
        W1.T.reshape(2, P, CTR_HID).transpose(1, 0, 2).reshape(P, 2 * CTR_HID)
    )
    b1 = np.asarray(inputs["b1"], np.float32)
    w2t = np.asarray(inputs["W2"], np.float32).T
    b2 = np.asarray(inputs["b2"], np.float32)
    Wa = np.asarray(inputs["Wa"], np.float32)
    WaP = Wa.reshape(R, IN, FD).transpose(0, 2, 1).reshape(R * FD, IN)
    wap = np.ascontiguousarray(
        WaP.reshape(2, P, IN).transpose(1, 0, 2).reshape(P, 2 * IN)
    ).astype(BF16)
    Wb = np.asarray(inputs["Wb"], np.float32) * SCALING
    WbP = Wb.reshape(R, OUT, FD).transpose(0, 2, 1).reshape(R * FD, OUT)
    wbp = np.ascontiguousarray(
        WbP.reshape(2, P, OUT).transpose(1, 0, 2).reshape(P, 2 * OUT)
    ).astype(BF16)
    wab = np.ascontiguousarray(np.concatenate([wap, wbp], axis=1))

    # packed gating constants [128, GPC]
    base = np.zeros((P, GPC), np.float32)
    base[0:BS, CTR0 : CTR0 + CTR_OUT] = ctr
    base[:, W1T0 : W1T0 + 2 * CTR_HID] = w1t
    base[0:CTR_HID, W2T0 : W2T0 + FD] = w2t
    base[0:CTR_HID, B10] = b1
    base[0:FD, B20] = b2
    # i16t[r, p] = 1 if p % 16 == r
    i16t = np.zeros((FD, P), np.float32)
    i16t[np.arange(P) % FD, np.arange(P)] = 1.0
    base[0:FD, I16T0 : I16T0 + P] = i16t
    # kron[p, c]: h = c//16, r = c%16; 1 iff r//8 == h and p//16 == r%8
    kron = np.zeros((P, 2 * FD), np.float32)
    for c in range(2 * FD):
        h, r = c // FD, c % FD
        if r // 8 == h:
            kron[(r % 8) * 16 : (r % 8 + 1) * 16, c] = 1.0
    base[:, KRON0 : KRON0 + 2 * FD] = kron
    base[0:BS, EPS0] = LN_EPS
    base[0:FD, ONE16] = 1.0
    base[0:1, ONE128 : ONE128 + P] = 1.0
    base[0:BS, GAM0 : GAM0 + CTR_OUT] = gam[None, :]
    base[0:BS, BET0 : BET0 + CTR_OUT] = bet[None, :]

    in_maps = []
    for c in range(BS):
        gpk = base.copy()
        onehot = np.zeros((BS,), np.float32)
        onehot[c] = 1.0
        gpk[0:FD, SEL0 : SEL0 + BS] = onehot[None, :]
        # xt[sb*128+p, ci*512+s] = x[c][sb*512+s, ci*128+p]
        xt = (
            x[c]
            .reshape(NSB, SBW, NC_I, P)
            .transpose(0, 3, 2, 1)
            .reshape(NSB * P, NC_I * SBW)
        )
        in_maps.append(dict(
            gpk=np.ascontiguousarray(gpk),
            wab=wab,
            xt=np.ascontiguousarray(xt).astype(BF16),
        ))
    return in_maps


def unscramble_y(y_dev):
    """y_dev [1024, 8192] bf16 -> y [2048, 4096] f32.

    y_dev[(sb*2+th)*128 + p, j*4096 + o] = y[(sb*4 + th*2 + j)*128 + p, o]
    """
    y = np.asarray(y_dev).reshape(NSB, 2, P, 2, OUT).transpose(0, 1, 3, 2, 4)
    return np.ascontiguousarray(y.reshape(SEQ, OUT)).astype(np.float32)


def get_compiled():
    global _COMPILED
    if _COMPILED is None:
        _COMPILED = build_program()
    return _COMPILED


def run(inputs, trace=False):
    from concourse.bass_utils import run_bass_kernel_spmd

    nc = get_compiled()
    in_maps = host_prep(inputs)
    res = run_bass_kernel_spmd(nc, in_maps, list(range(BS)), trace=trace)
    out = np.stack([unscramble_y(res.results[c]["y"]) for c in range(BS)], axis=0)
    return out, res


def kernel(**inputs) -> np.ndarray:
    out, _ = run(inputs, trace=False)
    return out


# revision 39
# speedup vs baseline: 1.0853x; 1.0853x over previous
"""Trainium2 Bass kernel for nn_Lorec (moe_routing LoRA-with-soft-routing).

Computation (per batch b):
  gate_b = softmax(MLP(LayerNorm(ctr[b])))                    [16]
  A_b[i,r] = sum_r' Wa[r*4096+i, r'] gate_b[r']               [4096,16]
  B_b[r,o] = sum_r' Wb[r*4096+o, r'] gate_b[r']               [16,4096]
  out[b] = (x[b] @ A_b) @ B_b * 2.0                           [2048,4096]

Sharding: data-parallel over bs=8 across 8 NeuronCores (one batch per core).
Gating replicated on every core (tiny); each core selects its own batch via a
one-hot `sel` input baked into the packed gating constants.

DMA-optimized: all bulk HBM traffic is bf16 (x in, y out, Wa/Wb), x is
pre-transposed on the host into xt[sb*128+p, c*512+s] = x[sb*512+s, c*128+p]
so mm1 consumes natural tiles with i on partitions (no device transposes).
Per-core HBM traffic ~36 MB (16 x + 16 y + 4 W).

PE-utilization tricks:
  - softmax denominator folded out: gate used UNNORMALIZED (exp only); the
    1/sum^2 factor is broadcast to [128,1] and folded into the PSUM->SBUF
    output copies (out is bilinear in gate).
  - mm1 (M=16): 2-way PE col-tiling -> psxa4 holds xa^T replicated at
    partition offsets 0/32 (c=0 uses a zero-padded full-width lhsT to
    initialize the whole PSUM bank).
  - mm2 (K=16): 2-way PE row-tiling -> t-tiles t,t+1 computed concurrently
    from xaT2/B_sb2 replicas at partition offsets 0/32.
  - PSUM->SBUF output copies rotate over ACT/DVE/GPSIMD.
  - all gating constants arrive in ONE packed [128,1204] f32 DMA.
"""

import sys

sys.path.insert(0, "/opt/trn_rl_repo")

import numpy as np
import ml_dtypes

BF16 = ml_dtypes.bfloat16

BS = 8
SEQ = 2048
IN = 4096
OUT = 4096
R = 16
CTR_OUT = 256
CTR_HID = 60
FD = 16  # FINAL_DIM
LN_EPS = 1e-5
SCALING = 2.0

P = 128
NSB = 4  # s-blocks per core
SBW = 512  # s-block width
NC_I = IN // P  # 32 i-chunks
NOB = OUT // 512  # 8 o-blocks

# packed gating tensor column offsets
CTR0 = 0
W1T0 = 256
W2T0 = 376
B10 = 392
B20 = 393
SEL0 = 394
I16T0 = 402
KRON0 = 530
EPS0 = 562
ONE16 = 563
ONE128 = 564
GAM0 = 692
BET0 = 948
GPC = 1204

_COMPILED = None


def build_program():
    import concourse.bass as bass
    import concourse.mybir as mybir
    from concourse import bacc
    from concourse.masks import make_identity
    from concourse.tile import TileContext

    f32 = mybir.dt.float32
    bf16 = mybir.dt.bfloat16
    AX = mybir.AxisListType.X
    ALU = mybir.AluOpType
    ACTF = mybir.ActivationFunctionType

    nc = bacc.Bacc("TRN2", target_bir_lowering=False, debug=False, num_devices=BS)

    xt_d = nc.dram_tensor("xt", [NSB * P, NC_I * SBW], bf16, kind="ExternalInput").ap()
    gpk_d = nc.dram_tensor("gpk", [P, GPC], f32, kind="ExternalInput").ap()
    wap_d = nc.dram_tensor("wap", [P, 2 * IN], bf16, kind="ExternalInput").ap()
    wbp_d = nc.dram_tensor("wbp", [P, 2 * OUT], bf16, kind="ExternalInput").ap()
    y_d = nc.dram_tensor("y", [2 * NSB * P, 2 * OUT], bf16, kind="ExternalOutput").ap()

    with TileContext(nc) as tc:
        with (
            tc.tile_pool(name="const", bufs=1) as const,
            tc.tile_pool(name="gp", bufs=1) as gp,
            tc.tile_pool(name="wpool", bufs=2) as wpool,
            tc.tile_pool(name="xpool", bufs=3) as xpool,
            tc.tile_pool(name="xapool", bufs=2) as xapool,
            tc.tile_pool(name="opool", bufs=3) as opool,
            tc.tile_pool(name="psxa_pool", bufs=2, space="PSUM") as psxa_pool,
            tc.tile_pool(name="pso_pool", bufs=5, space="PSUM") as pso_pool,
            tc.tile_pool(name="psg_pool", bufs=1, space="PSUM") as psg_pool,
        ):
            # ---- big-stream DMAs, queued on the sync (SP HWDGE) ring ----
            gpk = gp.tile([P, GPC], f32)
            nc.scalar.dma_start(out=gpk[:], in_=gpk_d[:])
            # each weight tensor as ONE 2 MB DMA (same queue positions as the
            # former 1 MB halves — queue order matters more than size)
            wapt = wpool.tile([P, 2 * IN], bf16, tag="wst")
            nc.sync.dma_start(out=wapt[:], in_=wap_d[:])
            waps = [wapt[:, h * IN : (h + 1) * IN] for h in range(2)]
            xsbs = {}
            xsb0 = xpool.tile([P, NC_I * SBW], bf16, tag="xsb", name="xsb0")
            nc.sync.dma_start(out=xsb0[:], in_=xt_d[0:P, :])
            xsbs[0] = xsb0
            wbpt = wpool.tile([P, 2 * OUT], bf16, tag="wst")
            nc.sync.dma_start(out=wbpt[:], in_=wbp_d[:])
            wbps = [wbpt[:, h * OUT : (h + 1) * OUT] for h in range(2)]
            xsb1 = xpool.tile([P, NC_I * SBW], bf16, tag="xsb", name="xsb1")
            nc.sync.dma_start(out=xsb1[:], in_=xt_d[P : 2 * P, :])
            xsbs[1] = xsb1

            ident = const.tile([P, P], f32)
            make_identity(nc, ident)

            # slices of the packed gating tile
            ctr = gpk[0:BS, CTR0 : CTR0 + CTR_OUT]
            gam = gpk[0:BS, GAM0 : GAM0 + CTR_OUT]
            bet = gpk[0:BS, BET0 : BET0 + CTR_OUT]
            w1t = gpk[0:P, W1T0 : W1T0 + 2 * CTR_HID]
            w2t = gpk[0:CTR_HID, W2T0 : W2T0 + FD]
            b1 = gpk[0:CTR_HID, B10 : B10 + 1]
            b2 = gpk[0:FD, B20 : B20 + 1]
            sel = gpk[0:FD, SEL0 : SEL0 + BS]
            i16t = gpk[0:FD, I16T0 : I16T0 + P]
            kron = gpk[0:P, KRON0 : KRON0 + 2 * FD]
            eps = gpk[0:BS, EPS0 : EPS0 + 1]
            one16 = gpk[0:FD, ONE16 : ONE16 + 1]
            one128 = gpk[0:1, ONE128 : ONE128 + P]

            # ---- LayerNorm on [8, 256] ----
            mean = gp.tile([BS, 1], f32)
            xc = gp.tile([BS, CTR_OUT], f32)
            sq = gp.tile([BS, CTR_OUT], f32)
            vs = gp.tile([BS, 1], f32)
            std = gp.tile([BS, 1], f32)
            rstd = gp.tile([BS, 1], f32)
            hh = gp.tile([BS, CTR_OUT], f32)
            # ops grouped by engine (DVE runs, then ACT runs) to minimize
            # cross-engine semaphore hops on this serial critical path
            nc.vector.tensor_reduce(mean[:], ctr, axis=AX, op=ALU.add)
            nc.vector.tensor_scalar_mul(mean[:], mean[:], 1.0 / CTR_OUT)
            nc.vector.tensor_scalar_sub(xc[:], ctr, mean[:])
            nc.scalar.activation(sq[:], xc[:], ACTF.Square, accum_out=vs[:])
            nc.scalar.activation(std[:], vs[:], ACTF.Sqrt, bias=eps, scale=1.0 / CTR_OUT)
            nc.vector.reciprocal(rstd[:], std[:])
            nc.vector.tensor_scalar_mul(hh[:], xc[:], rstd[:])
            nc.vector.tensor_mul(hh[:], hh[:], gam)
            nc.vector.tensor_add(hh[:], hh[:], bet)

            # ---- hT [256->2x128, 8] via PE transpose ----
            hT = gp.tile([P, 2 * BS], f32)
            for h in range(2):
                pt = psg_pool.tile([P, BS], f32, tag="psg_small")
                nc.tensor.transpose(pt[:], hh[:, h * P : (h + 1) * P], ident[0:BS, 0:BS])
                nc.scalar.copy(hT[:, h * BS : (h + 1) * BS], pt[:])

            # ---- h1T = relu(W1 @ h + b1) -> [60, 8] ----
            ph1 = psg_pool.tile([CTR_HID, BS], f32, tag="psg_small")
            for h in range(2):
                nc.tensor.matmul(
                    ph1[:], w1t[:, h * CTR_HID : (h + 1) * CTR_HID],
                    hT[:, h * BS : (h + 1) * BS], start=(h == 0), stop=(h == 1),
                )
            h1T = gp.tile([CTR_HID, BS], f32)
            nc.scalar.activation(h1T[:], ph1[:], ACTF.Relu, bias=b1)

            # ---- logitsT = W2 @ h1 + b2 -> [16, 8] ----
            plog = psg_pool.tile([FD, BS], f32, tag="psg_small")
            nc.tensor.matmul(plog[:], w2t, h1T[:], start=True, stop=True)
            logitsT = gp.tile([FD, BS], f32)
            nc.scalar.activation(logitsT[:], plog[:], ACTF.Identity, bias=b2)

            # ---- select own batch, unnormalized gate e = exp(logit_b) ----
            lsel = gp.tile([FD, BS], f32)
            logit_b = gp.tile([FD, 1], f32)
            nc.vector.tensor_mul(lsel[:], logitsT[:], sel)
            nc.vector.tensor_reduce(logit_b[:], lsel[:], axis=AX, op=ALU.add)
            eb = gp.tile([FD, 1], f32)
            nc.scalar.activation(eb[:], logit_b[:], ACTF.Exp)

            # ---- G = I_16 kron e, layout [128, 2*16] bf16 via mask*bcast ----
            pgt = psg_pool.tile([P, 1], f32, tag="psg_small")
            nc.tensor.matmul(pgt[:], i16t, eb[:], start=True, stop=True)
            gtile = gp.tile([P, 1], f32)
            nc.scalar.copy(gtile[:], pgt[:])
            G = gp.tile([P, 2 * FD], bf16)
            nc.vector.tensor_scalar_mul(G[:], kron, gtile[:])
            # Gz: G halves zero-padded to 32 columns each. All down-stream
            # matmuls use 32-wide stationary operands so every PE col/row
            # strip is nominally active (keeps the HAM clock-gate at 8/8).
            Gz = gp.tile([P, 2 * 32], bf16)
            nc.gpsimd.memset(Gz[:], 0.0)
            for h in range(2):
                nc.scalar.copy(Gz[:, h * 32 : h * 32 + FD], G[:, h * FD : (h + 1) * FD])

            # ---- rsq = 1/sum(e)^2 broadcast to [128,1] (off critical path) ----
            psum1 = psg_pool.tile([1, 1], f32, tag="psg_small")
            nc.tensor.matmul(psum1[:], one16, eb[:], start=True, stop=True)
            ssum = gp.tile([1, 1], f32)
            nc.vector.tensor_copy(ssum[:], psum1[:])
            rs = gp.tile([1, 1], f32)
            nc.vector.reciprocal(rs[:], ssum[:])
            rs2 = gp.tile([1, 1], f32)
            nc.vector.tensor_mul(rs2[:], rs[:], rs[:])
            prsq = psg_pool.tile([P, 1], f32, tag="psg_small")
            nc.tensor.matmul(prsq[:], one128, rs2[:], start=True, stop=True)
            rsq = gp.tile([P, 1], f32)
            nc.scalar.copy(rsq[:], prsq[:])

            # ---- A-gen: A_sb[p, c*32+r] = A[c*128+p, r] (bf16, zero-padded to
            # 32-col blocks; padding comes from Gz's zero columns) ----
            A_sb = gp.tile([P, NC_I * 32], bf16)
            for half in range(2):
                psA = psxa_pool.tile([P, 512], f32, tag="psmm", name="psA")
                for cl in range(NC_I // 2):
                    c = half * (NC_I // 2) + cl
                    for h in range(2):
                        nc.tensor.matmul(
                            psA[:, cl * 32 : (cl + 1) * 32],
                            waps[h][:, c * P : (c + 1) * P],
                            Gz[:, h * 32 : (h + 1) * 32],
                            start=(h == 0), stop=(h == 1),
                        )
                nc.scalar.copy(A_sb[:, half * 512 : (half + 1) * 512], psA[:])

            # ---- B-gen: B_sb4 [128, 4096] bf16 = B replicated at 0/32/64/96 ----
            # (emitted after mm1(sb0) in the main loop so the PE doesn't stall
            # on the later-arriving wbp DMAs before starting mm1)
            B_sb4 = gp.tile([P, OUT], bf16)

            def emit_bgen():
                for ob in range(NOB):
                    psB = psxa_pool.tile([P, 512], f32, tag="psmm", name="psB")
                    for h in range(2):
                        for j in range(4):
                            nc.tensor.matmul(
                                psB[32 * j : 32 * j + 32, :],
                                Gz[:, h * 32 : (h + 1) * 32],
                                wbps[h][:, ob * 512 : (ob + 1) * 512],
                                start=(h == 0), stop=(h == 1), skip_group_check=True,
                                tile_position=(0, 32 * j),
                            )
                    if ob % 2 == 0:
                        nc.scalar.copy(B_sb4[:, ob * 512 : (ob + 1) * 512], psB[:])
                    else:
                        nc.vector.tensor_copy(B_sb4[:, ob * 512 : (ob + 1) * 512], psB[:])

            # ---- main loop over s-blocks ----
            # mm2 rounds of s-block sb-1 are interleaved into mm1(sb)'s
            # instruction stream so PSUM->SBUF copies drain in parallel and
            # y DMAs spread through the loop instead of bursting at the end.
            # Half-s-block pipelining: 8 stages of 256 seq positions. Each
            # stage's mm2 is only 4 rounds (4-way: 2 t-tiles x 2 ob-halves),
            # interleaved into the next stage's mm1, so the un-overlapped
            # drain at the end of the kernel is one half-block.
            HB = SBW // 2  # 256

            def emit_mm2_round(pd):
                r = pd["ops"].pop(0)  # 0..3; round r completes obs {2r, 2r+1}
                xa = pd["xaT4"]
                psos = []
                for g in range(4):
                    t = g % 2
                    ob = 2 * r + g // 2
                    pso = pso_pool.tile([P, 512], f32, tag="pso")
                    nc.tensor.matmul(
                        pso[:],
                        xa[32 * g : 32 * g + 32, t * P : (t + 1) * P],
                        B_sb4[32 * g : 32 * g + 32, ob * 512 : (ob + 1) * 512],
                        start=True, stop=True,
                        tile_position=(32 * g, 0),
                    )
                    psos.append(pso)
                for g in range(4):
                    t = g % 2
                    ob = 2 * r + g // 2
                    dst = pd["out"][:, t * OUT + ob * 512 : t * OUT + (ob + 1) * 512]
                    if g % 2 == 0:
                        nc.scalar.activation(dst, psos[g][:], ACTF.Copy, scale=rsq[:])
                    else:
                        nc.vector.tensor_scalar_mul(dst, psos[g][:], rsq[:])
                i = pd["h"]
                if pd["h"] == 2 * NSB - 1:
                    # final stage: release y per round (256 KB pieces) so the
                    # end-of-kernel drain is just the last ob pair
                    c0 = r * 1024
                    for t in range(2):
                        nc.scalar.dma_start(
                            out=y_d[i * P : (i + 1) * P, t * OUT + c0 : t * OUT + c0 + 1024],
                            in_=pd["out"][:, t * OUT + c0 : t * OUT + c0 + 1024],
                        )
                elif pd["h"] == 2 * NSB - 2:
                    # next-to-last stage: 512 KB halves at rounds 1 and 3
                    if r == 1 or r == 3:
                        c0 = 0 if r == 1 else 2048
                        for t in range(2):
                            nc.scalar.dma_start(
                                out=y_d[i * P : (i + 1) * P, t * OUT + c0 : t * OUT + c0 + 2048],
                                in_=pd["out"][:, t * OUT + c0 : t * OUT + c0 + 2048],
                            )
                elif not pd["ops"]:
                    nc.scalar.dma_start(
                        out=y_d[i * P : (i + 1) * P, :], in_=pd["out"][:]
                    )

            pend = None
            for h in range(2 * NSB):
                sbq, s0 = h // 2, (h % 2) * HB
                if h % 2 == 0 and sbq + 2 < NSB:
                    nsb = sbq + 2
                    xsb_n = xpool.tile([P, NC_I * SBW], bf16, tag="xsb")
                    nc.sync.dma_start(
                        out=xsb_n[:], in_=xt_d[nsb * P : (nsb + 1) * P, :]
                    )
                    xsbs[nsb] = xsb_n
                xsb = xsbs[sbq]

                # mm1: xa^T replicated at partition offsets 0/32/64/96, each
                # group 32 cols wide (16 real + 16 zero-pad) -> full PE width
                psxa4 = psxa_pool.tile([P, HB], f32, tag="psmm", name="psxa4")
                for c in range(NC_I):
                    for j in range(4):
                        nc.tensor.matmul(
                            psxa4[32 * j : 32 * j + 32, :],
                            A_sb[:, c * 32 : (c + 1) * 32],
                            xsb[:, c * SBW + s0 : c * SBW + s0 + HB],
                            start=(c == 0), stop=(c == NC_I - 1), skip_group_check=True,
                            tile_position=(0, 32 * j),
                        )
                    if c % 8 == 4 and pend is not None and pend["ops"]:
                        emit_mm2_round(pend)
                xaT4 = xapool.tile([P, HB], bf16, tag="xaT")
                nc.vector.tensor_copy(xaT4[:], psxa4[:])
                while pend is not None and pend["ops"]:
                    emit_mm2_round(pend)
                if h == 0:
                    emit_bgen()
                pend = dict(
                    h=h,
                    xaT4=xaT4,
                    out=opool.tile([P, 2 * OUT], bf16, tag="osb", name="outT"),
                    ops=list(range(4)),
                )
            while pend["ops"]:
                emit_mm2_round(pend)

    nc.compile()
    return nc


def host_prep(inputs):
    """Build per-core and shared input arrays from the full problem inputs."""
    x = np.asarray(inputs["x"], np.float32)
    ctr = np.asarray(inputs["ctr_hidden_states"], np.float32)
    gam = np.asarray(inputs["ln_gamma"], np.float32)
    bet = np.asarray(inputs["ln_beta"], np.float32)
    W1 = np.asarray(inputs["W1"], np.float32)
    w1t = np.ascontiguousarray(
        W1.T.reshape(2, P, CTR_HID).transpose(1, 0, 2).reshape(P, 2 * CTR_HID)
    )
    b1 = np.asarray(inputs["b1"], np.float32)
    w2t = np.asarray(inputs["W2"], np.float32).T
    b2 = np.asarray(inputs["b2"], np.float32)
    Wa = np.asarray(inputs["Wa"], np.float32)
    WaP = Wa.reshape(R, IN, FD).transpose(0, 2, 1).reshape(R * FD, IN)
    wap = np.ascontiguousarray(
        WaP.reshape(2, P, IN).transpose(1, 0, 2).reshape(P, 2 * IN)
    ).astype(BF16)
    Wb = np.asarray(inputs["Wb"], np.float32) * SCALING
    WbP = Wb.reshape(R, OUT, FD).transpose(0, 2, 1).reshape(R * FD, OUT)
    wbp = np.ascontiguousarray(
        WbP.reshape(2, P, OUT).transpose(1, 0, 2).reshape(P, 2 * OUT)
    ).astype(BF16)

    # packed gating constants [128, GPC]
    base = np.zeros((P, GPC), np.float32)
    base[0:BS, CTR0 : CTR0 + CTR_OUT] = ctr
    base[:, W1T0 : W1T0 + 2 * CTR_HID] = w1t
    base[0:CTR_HID, W2T0 : W2T0 + FD] = w2t
    base[0:CTR_HID, B10] = b1
    base[0:FD, B20] = b2
    # i16t[r, p] = 1 if p % 16 == r
    i16t = np.zeros((FD, P), np.float32)
    i16t[np.arange(P) % FD, np.arange(P)] = 1.0
    base[0:FD, I16T0 : I16T0 + P] = i16t
    # kron[p, c]: h = c//16, r = c%16; 1 iff r//8 == h and p//16 == r%8
    kron = np.zeros((P, 2 * FD), np.float32)
    for c in range(2 * FD):
        h, r = c // FD, c % FD
        if r // 8 == h:
            kron[(r % 8) * 16 : (r % 8 + 1) * 16, c] = 1.0
    base[:, KRON0 : KRON0 + 2 * FD] = kron
    base[0:BS, EPS0] = LN_EPS
    base[0:FD, ONE16] = 1.0
    base[0:1, ONE128 : ONE128 + P] = 1.0
    base[0:BS, GAM0 : GAM0 + CTR_OUT] = gam[None, :]
    base[0:BS, BET0 : BET0 + CTR_OUT] = bet[None, :]

    in_maps = []
    for c in range(BS):
        gpk = base.copy()
        onehot = np.zeros((BS,), np.float32)
        onehot[c] = 1.0
        gpk[0:FD, SEL0 : SEL0 + BS] = onehot[None, :]
        # xt[sb*128+p, ci*512+s] = x[c][sb*512+s, ci*128+p]
        xt = (
            x[c]
            .reshape(NSB, SBW, NC_I, P)
            .transpose(0, 3, 2, 1)
            .reshape(NSB * P, NC_I * SBW)
        )
        in_maps.append(dict(
            gpk=np.ascontiguousarray(gpk),
            wap=wap, wbp=wbp,
            xt=np.ascontiguousarray(xt).astype(BF16),
        ))
    return in_maps


def unscramble_y(y_dev):
    """y_dev [1024, 8192] bf16 -> y [2048, 4096] f32.

    y_dev[(sb*2+th)*128 + p, j*4096 + o] = y[(sb*4 + th*2 + j)*128 + p, o]
    """
    y = np.asarray(y_dev).reshape(NSB, 2, P, 2, OUT).transpose(0, 1, 3, 2, 4)
    return np.ascontiguousarray(y.reshape(SEQ, OUT)).astype(np.float32)


def get_compiled():
    global _COMPILED
    if _COMPILED is None:
        _COMPILED = build_program()
    return _COMPILED


def run(inputs, trace=False):
    from concourse.bass_utils import run_bass_kernel_spmd

    nc = get_compiled()
    in_maps = host_prep(inputs)
    res = run_bass_kernel_spmd(nc, in_maps, list(range(BS)), trace=trace)
    out = np.stack([unscramble_y(res.results[c]["y"]) for c in range(BS)], axis=0)
    return out, res


def kernel(**inputs) -> np.ndarray:
    out, _ = run(inputs, trace=False)
    return out


# revision 40
# speedup vs baseline: 1.1236x; 1.0353x over previous
"""Trainium2 Bass kernel for nn_Lorec (moe_routing LoRA-with-soft-routing).

Computation (per batch b):
  gate_b = softmax(MLP(LayerNorm(ctr[b])))                    [16]
  A_b[i,r] = sum_r' Wa[r*4096+i, r'] gate_b[r']               [4096,16]
  B_b[r,o] = sum_r' Wb[r*4096+o, r'] gate_b[r']               [16,4096]
  out[b] = (x[b] @ A_b) @ B_b * 2.0                           [2048,4096]

Sharding: data-parallel over bs=8 across 8 NeuronCores (one batch per core).
Gating replicated on every core (tiny); each core selects its own batch via a
one-hot `sel` input baked into the packed gating constants.

DMA-optimized: all bulk HBM traffic is bf16 (x in, y out, Wa/Wb), x is
pre-transposed on the host into xt[sb*128+p, c*512+s] = x[sb*512+s, c*128+p]
so mm1 consumes natural tiles with i on partitions (no device transposes).
Per-core HBM traffic ~36 MB (16 x + 16 y + 4 W).

PE-utilization tricks:
  - softmax denominator folded out: gate used UNNORMALIZED (exp only); the
    1/sum^2 factor is broadcast to [128,1] and folded into the PSUM->SBUF
    output copies (out is bilinear in gate).
  - mm1 (M=16): 2-way PE col-tiling -> psxa4 holds xa^T replicated at
    partition offsets 0/32 (c=0 uses a zero-padded full-width lhsT to
    initialize the whole PSUM bank).
  - mm2 (K=16): 2-way PE row-tiling -> t-tiles t,t+1 computed concurrently
    from xaT2/B_sb2 replicas at partition offsets 0/32.
  - PSUM->SBUF output copies rotate over ACT/DVE/GPSIMD.
  - all gating constants arrive in ONE packed [128,1204] f32 DMA.
"""

import sys

sys.path.insert(0, "/opt/trn_rl_repo")

import numpy as np
import ml_dtypes

BF16 = ml_dtypes.bfloat16

BS = 8
SEQ = 2048
IN = 4096
OUT = 4096
R = 16
CTR_OUT = 256
CTR_HID = 60
FD = 16  # FINAL_DIM
LN_EPS = 1e-5
SCALING = 2.0

P = 128
NSB = 4  # s-blocks per core
SBW = 512  # s-block width
NC_I = IN // P  # 32 i-chunks
NOB = OUT // 512  # 8 o-blocks

# packed gating tensor column offsets
CTR0 = 0
W1T0 = 256
W2T0 = 376
B10 = 392
B20 = 393
SEL0 = 394
I16T0 = 402
KRON0 = 530
EPS0 = 562
ONE16 = 563
ONE128 = 564
GAM0 = 692
BET0 = 948
GPC = 1204

_COMPILED = None


def build_program():
    import concourse.bass as bass
    import concourse.mybir as mybir
    from concourse import bacc
    from concourse.masks import make_identity
    from concourse.tile import TileContext

    f32 = mybir.dt.float32
    bf16 = mybir.dt.bfloat16
    AX = mybir.AxisListType.X
    ALU = mybir.AluOpType
    ACTF = mybir.ActivationFunctionType

    nc = bacc.Bacc("TRN2", target_bir_lowering=False, debug=False, num_devices=BS)

    xt_d = nc.dram_tensor("xt", [NSB * P, NC_I * SBW], bf16, kind="ExternalInput").ap()
    gpk_d = nc.dram_tensor("gpk", [P, GPC], f32, kind="ExternalInput").ap()
    wap_d = nc.dram_tensor("wap", [P, 2 * IN], bf16, kind="ExternalInput").ap()
    wbp_d = nc.dram_tensor("wbp", [P, 2 * OUT], bf16, kind="ExternalInput").ap()
    y_d = nc.dram_tensor("y", [2 * NSB * P, 2 * OUT], bf16, kind="ExternalOutput").ap()

    with TileContext(nc) as tc:
        with (
            tc.tile_pool(name="const", bufs=1) as const,
            tc.tile_pool(name="gp", bufs=1) as gp,
            tc.tile_pool(name="wpool", bufs=4) as wpool,
            tc.tile_pool(name="xpool", bufs=3) as xpool,
            tc.tile_pool(name="xapool", bufs=2) as xapool,
            tc.tile_pool(name="opool", bufs=3) as opool,
            tc.tile_pool(name="psxa_pool", bufs=2, space="PSUM") as psxa_pool,
            tc.tile_pool(name="pso_pool", bufs=5, space="PSUM") as pso_pool,
            tc.tile_pool(name="psg_pool", bufs=1, space="PSUM") as psg_pool,
        ):
            # ---- big-stream DMAs, queued on the sync (SP HWDGE) ring ----
            gpk = gp.tile([P, GPC], f32)
            nc.scalar.dma_start(out=gpk[:], in_=gpk_d[:])
            waps = []
            for h in range(2):
                wt = wpool.tile([P, IN], bf16, tag="wst")
                nc.sync.dma_start(out=wt[:], in_=wap_d[:, h * IN : (h + 1) * IN])
                waps.append(wt)
            xsbs = {}
            xsb0 = xpool.tile([P, NC_I * SBW], bf16, tag="xsb", name="xsb0")
            nc.sync.dma_start(out=xsb0[:], in_=xt_d[0:P, :])
            xsbs[0] = xsb0
            wbps = []
            for h in range(2):
                wt = wpool.tile([P, OUT], bf16, tag="wst")
                nc.sync.dma_start(out=wt[:], in_=wbp_d[:, h * OUT : (h + 1) * OUT])
                wbps.append(wt)
            xsb1 = xpool.tile([P, NC_I * SBW], bf16, tag="xsb", name="xsb1")
            nc.sync.dma_start(out=xsb1[:], in_=xt_d[P : 2 * P, :])
            xsbs[1] = xsb1

            ident = const.tile([P, P], f32)
            make_identity(nc, ident)

            # slices of the packed gating tile
            ctr = gpk[0:BS, CTR0 : CTR0 + CTR_OUT]
            gam = gpk[0:BS, GAM0 : GAM0 + CTR_OUT]
            bet = gpk[0:BS, BET0 : BET0 + CTR_OUT]
            w1t = gpk[0:P, W1T0 : W1T0 + 2 * CTR_HID]
            w2t = gpk[0:CTR_HID, W2T0 : W2T0 + FD]
            b1 = gpk[0:CTR_HID, B10 : B10 + 1]
            b2 = gpk[0:FD, B20 : B20 + 1]
            sel = gpk[0:FD, SEL0 : SEL0 + BS]
            i16t = gpk[0:FD, I16T0 : I16T0 + P]
            kron = gpk[0:P, KRON0 : KRON0 + 2 * FD]
            eps = gpk[0:BS, EPS0 : EPS0 + 1]
            one16 = gpk[0:FD, ONE16 : ONE16 + 1]
            one128 = gpk[0:1, ONE128 : ONE128 + P]

            # ---- LayerNorm on [8, 256] ----
            mean = gp.tile([BS, 1], f32)
            xc = gp.tile([BS, CTR_OUT], f32)
            sq = gp.tile([BS, CTR_OUT], f32)
            vs = gp.tile([BS, 1], f32)
            std = gp.tile([BS, 1], f32)
            rstd = gp.tile([BS, 1], f32)
            hh = gp.tile([BS, CTR_OUT], f32)
            # ops grouped by engine (DVE runs, then ACT runs) to minimize
            # cross-engine semaphore hops on this serial critical path
            nc.vector.tensor_reduce(mean[:], ctr, axis=AX, op=ALU.add)
            nc.vector.tensor_scalar_mul(mean[:], mean[:], 1.0 / CTR_OUT)
            nc.vector.tensor_scalar_sub(xc[:], ctr, mean[:])
            nc.scalar.activation(sq[:], xc[:], ACTF.Square, accum_out=vs[:])
            nc.scalar.activation(std[:], vs[:], ACTF.Sqrt, bias=eps, scale=1.0 / CTR_OUT)
            nc.vector.reciprocal(rstd[:], std[:])
            nc.vector.tensor_scalar_mul(hh[:], xc[:], rstd[:])
            nc.vector.tensor_mul(hh[:], hh[:], gam)
            nc.vector.tensor_add(hh[:], hh[:], bet)

            # ---- hT [256->2x128, 8] via PE transpose ----
            hT = gp.tile([P, 2 * BS], f32)
            for h in range(2):
                pt = psg_pool.tile([P, BS], f32, tag="psg_small")
                nc.tensor.transpose(pt[:], hh[:, h * P : (h + 1) * P], ident[0:BS, 0:BS])
                nc.scalar.copy(hT[:, h * BS : (h + 1) * BS], pt[:])

            # ---- h1T = relu(W1 @ h + b1) -> [60, 8] ----
            ph1 = psg_pool.tile([CTR_HID, BS], f32, tag="psg_small")
            for h in range(2):
                nc.tensor.matmul(
                    ph1[:], w1t[:, h * CTR_HID : (h + 1) * CTR_HID],
                    hT[:, h * BS : (h + 1) * BS], start=(h == 0), stop=(h == 1),
                )
            h1T = gp.tile([CTR_HID, BS], f32)
            nc.scalar.activation(h1T[:], ph1[:], ACTF.Relu, bias=b1)

            # ---- logitsT = W2 @ h1 + b2 -> [16, 8] ----
            plog = psg_pool.tile([FD, BS], f32, tag="psg_small")
            nc.tensor.matmul(plog[:], w2t, h1T[:], start=True, stop=True)
            logitsT = gp.tile([FD, BS], f32)
            nc.scalar.activation(logitsT[:], plog[:], ACTF.Identity, bias=b2)

            # ---- select own batch, unnormalized gate e = exp(logit_b) ----
            lsel = gp.tile([FD, BS], f32)
            logit_b = gp.tile([FD, 1], f32)
            nc.vector.tensor_mul(lsel[:], logitsT[:], sel)
            nc.vector.tensor_reduce(logit_b[:], lsel[:], axis=AX, op=ALU.add)
            eb = gp.tile([FD, 1], f32)
            nc.scalar.activation(eb[:], logit_b[:], ACTF.Exp)

            # ---- G = I_16 kron e, layout [128, 2*16] bf16 via mask*bcast ----
            pgt = psg_pool.tile([P, 1], f32, tag="psg_small")
            nc.tensor.matmul(pgt[:], i16t, eb[:], start=True, stop=True)
            gtile = gp.tile([P, 1], f32)
            nc.scalar.copy(gtile[:], pgt[:])
            G = gp.tile([P, 2 * FD], bf16)
            nc.vector.tensor_scalar_mul(G[:], kron, gtile[:])
            # Gz: G halves zero-padded to 32 columns each. All down-stream
            # matmuls use 32-wide stationary operands so every PE col/row
            # strip is nominally active (keeps the HAM clock-gate at 8/8).
            Gz = gp.tile([P, 2 * 32], bf16)
            nc.gpsimd.memset(Gz[:], 0.0)
            for h in range(2):
                nc.scalar.copy(Gz[:, h * 32 : h * 32 + FD], G[:, h * FD : (h + 1) * FD])

            # ---- rsq = 1/sum(e)^2 broadcast to [128,1] (off critical path) ----
            psum1 = psg_pool.tile([1, 1], f32, tag="psg_small")
            nc.tensor.matmul(psum1[:], one16, eb[:], start=True, stop=True)
            ssum = gp.tile([1, 1], f32)
            nc.vector.tensor_copy(ssum[:], psum1[:])
            rs = gp.tile([1, 1], f32)
            nc.vector.reciprocal(rs[:], ssum[:])
            rs2 = gp.tile([1, 1], f32)
            nc.vector.tensor_mul(rs2[:], rs[:], rs[:])
            prsq = psg_pool.tile([P, 1], f32, tag="psg_small")
            nc.tensor.matmul(prsq[:], one128, rs2[:], start=True, stop=True)
            rsq = gp.tile([P, 1], f32)
            nc.scalar.copy(rsq[:], prsq[:])

            # ---- A-gen: A_sb[p, c*32+r] = A[c*128+p, r] (bf16, zero-padded to
            # 32-col blocks; padding comes from Gz's zero columns) ----
            A_sb = gp.tile([P, NC_I * 32], bf16)
            for half in range(2):
                psA = psxa_pool.tile([P, 512], f32, tag="psmm", name="psA")
                for cl in range(NC_I // 2):
                    c = half * (NC_I // 2) + cl
                    for h in range(2):
                        nc.tensor.matmul(
                            psA[:, cl * 32 : (cl + 1) * 32],
                            waps[h][:, c * P : (c + 1) * P],
                            Gz[:, h * 32 : (h + 1) * 32],
                            start=(h == 0), stop=(h == 1),
                        )
                nc.scalar.copy(A_sb[:, half * 512 : (half + 1) * 512], psA[:])

            # ---- B-gen: B_sb4 [128, 4096] bf16 = B replicated at 0/32/64/96 ----
            # (emitted after mm1(sb0) in the main loop so the PE doesn't stall
            # on the later-arriving wbp DMAs before starting mm1)
            B_sb4 = gp.tile([P, OUT], bf16)

            def emit_bgen():
                for ob in range(NOB):
                    psB = psxa_pool.tile([P, 512], f32, tag="psmm", name="psB")
                    for h in range(2):
                        for j in range(4):
                            nc.tensor.matmul(
                                psB[32 * j : 32 * j + 32, :],
                                Gz[:, h * 32 : (h + 1) * 32],
                                wbps[h][:, ob * 512 : (ob + 1) * 512],
                                start=(h == 0), stop=(h == 1), skip_group_check=True,
                                tile_position=(0, 32 * j),
                            )
                    if ob % 2 == 0:
                        nc.scalar.copy(B_sb4[:, ob * 512 : (ob + 1) * 512], psB[:])
                    else:
                        nc.vector.tensor_copy(B_sb4[:, ob * 512 : (ob + 1) * 512], psB[:])

            # ---- main loop over s-blocks ----
            # mm2 rounds of s-block sb-1 are interleaved into mm1(sb)'s
            # instruction stream so PSUM->SBUF copies drain in parallel and
            # y DMAs spread through the loop instead of bursting at the end.
            # Half-s-block pipelining: 8 stages of 256 seq positions. Each
            # stage's mm2 is only 4 rounds (4-way: 2 t-tiles x 2 ob-halves),
            # interleaved into the next stage's mm1, so the un-overlapped
            # drain at the end of the kernel is one half-block.
            HB = SBW // 2  # 256

            def emit_mm2_round(pd):
                r = pd["ops"].pop(0)  # 0..3; round r completes obs {2r, 2r+1}
                xa = pd["xaT4"]
                psos = []
                for g in range(4):
                    t = g % 2
                    ob = 2 * r + g // 2
                    pso = pso_pool.tile([P, 512], f32, tag="pso")
                    nc.tensor.matmul(
                        pso[:],
                        xa[32 * g : 32 * g + 32, t * P : (t + 1) * P],
                        B_sb4[32 * g : 32 * g + 32, ob * 512 : (ob + 1) * 512],
                        start=True, stop=True,
                        tile_position=(32 * g, 0),
                    )
                    psos.append(pso)
                for g in range(4):
                    t = g % 2
                    ob = 2 * r + g // 2
                    dst = pd["out"][:, t * OUT + ob * 512 : t * OUT + (ob + 1) * 512]
                    if g % 2 == 0:
                        nc.scalar.activation(dst, psos[g][:], ACTF.Copy, scale=rsq[:])
                    else:
                        nc.vector.tensor_scalar_mul(dst, psos[g][:], rsq[:])
                i = pd["h"]
                if pd["h"] == 2 * NSB - 1:
                    # final stage: release y in 512 KB halves as soon as each
                    # ob-half completes, shrinking the end-of-kernel drain
                    if r == 1 or r == 3:
                        c0 = 0 if r == 1 else 2048
                        for t in range(2):
                            nc.scalar.dma_start(
                                out=y_d[i * P : (i + 1) * P, t * OUT + c0 : t * OUT + c0 + 2048],
                                in_=pd["out"][:, t * OUT + c0 : t * OUT + c0 + 2048],
                            )
                elif not pd["ops"]:
                    nc.scalar.dma_start(
                        out=y_d[i * P : (i + 1) * P, :], in_=pd["out"][:]
                    )

            pend = None
            for h in range(2 * NSB):
                sbq, s0 = h // 2, (h % 2) * HB
                if h % 2 == 0 and sbq + 2 < NSB:
                    nsb = sbq + 2
                    xsb_n = xpool.tile([P, NC_I * SBW], bf16, tag="xsb")
                    nc.sync.dma_start(
                        out=xsb_n[:], in_=xt_d[nsb * P : (nsb + 1) * P, :]
                    )
                    xsbs[nsb] = xsb_n
                xsb = xsbs[sbq]

                # mm1: xa^T replicated at partition offsets 0/32/64/96, each
                # group 32 cols wide (16 real + 16 zero-pad) -> full PE width
                psxa4 = psxa_pool.tile([P, HB], f32, tag="psmm", name="psxa4")
                for c in range(NC_I):
                    for j in range(4):
                        nc.tensor.matmul(
                            psxa4[32 * j : 32 * j + 32, :],
                            A_sb[:, c * 32 : (c + 1) * 32],
                            xsb[:, c * SBW + s0 : c * SBW + s0 + HB],
                            start=(c == 0), stop=(c == NC_I - 1), skip_group_check=True,
                            tile_position=(0, 32 * j),
                        )
                    if c % 8 == 4 and pend is not None and pend["ops"]:
                        emit_mm2_round(pend)
                xaT4 = xapool.tile([P, HB], bf16, tag="xaT")
                nc.vector.tensor_copy(xaT4[:], psxa4[:])
                while pend is not None and pend["ops"]:
                    emit_mm2_round(pend)
                if h == 0:
                    emit_bgen()
                pend = dict(
                    h=h,
                    xaT4=xaT4,
                    out=opool.tile([P, 2 * OUT], bf16, tag="osb", name="outT"),
                    ops=list(range(4)),
                )
            while pend["ops"]:
                emit_mm2_round(pend)

    nc.compile()
    return nc


def host_prep(inputs):
    """Build per-core and shared input arrays from the full problem inputs."""
    x = np.asarray(inputs["x"], np.float32)
    ctr = np.asarray(inputs["ctr_hidden_states"], np.float32)
    gam = np.asarray(inputs["ln_gamma"], np.float32)
    bet = np.asarray(inputs["ln_beta"], np.float32)
    W1 = np.asarray(inputs["W1"], np.float32)
    w1t = np.ascontiguousarray(
        W1.T.reshape(2, P, CTR_HID).transpose(1, 0, 2).reshape(P, 2 * CTR_HID)
    )
    b1 = np.asarray(inputs["b1"], np.float32)
    w2t = np.asarray(inputs["W2"], np.float32).T
    b2 = np.asarray(inputs["b2"], np.float32)
    Wa = np.asarray(inputs["Wa"], np.float32)
    WaP = Wa.reshape(R, IN, FD).transpose(0, 2, 1).reshape(R * FD, IN)
    wap = np.ascontiguousarray(
        WaP.reshape(2, P, IN).transpose(1, 0, 2).reshape(P, 2 * IN)
    ).astype(BF16)
    Wb = np.asarray(inputs["Wb"], np.float32) * SCALING
    WbP = Wb.reshape(R, OUT, FD).transpose(0, 2, 1).reshape(R * FD, OUT)
    wbp = np.ascontiguousarray(
        WbP.reshape(2, P, OUT).transpose(1, 0, 2).reshape(P, 2 * OUT)
    ).astype(BF16)

    # packed gating constants [128, GPC]
    base = np.zeros((P, GPC), np.float32)
    base[0:BS, CTR0 : CTR0 + CTR_OUT] = ctr
    base[:, W1T0 : W1T0 + 2 * CTR_HID] = w1t
    base[0:CTR_HID, W2T0 : W2T0 + FD] = w2t
    base[0:CTR_HID, B10] = b1
    base[0:FD, B20] = b2
    # i16t[r, p] = 1 if p % 16 == r
    i16t = np.zeros((FD, P), np.float32)
    i16t[np.arange(P) % FD, np.arange(P)] = 1.0
    base[0:FD, I16T0 : I16T0 + P] = i16t
    # kron[p, c]: h = c//16, r = c%16; 1 iff r//8 == h and p//16 == r%8
    kron = np.zeros((P, 2 * FD), np.float32)
    for c in range(2 * FD):
        h, r = c // FD, c % FD
        if r // 8 == h:
            kron[(r % 8) * 16 : (r % 8 + 1) * 16, c] = 1.0
    base[:, KRON0 : KRON0 + 2 * FD] = kron
    base[0:BS, EPS0] = LN_EPS
    base[0:FD, ONE16] = 1.0
    base[0:1, ONE128 : ONE128 + P] = 1.0
    base[0:BS, GAM0 : GAM0 + CTR_OUT] = gam[None, :]
    base[0:BS, BET0 : BET0 + CTR_OUT] = bet[None, :]

    in_maps = []
    for c in range(BS):
        gpk = base.copy()
        onehot = np.zeros((BS,), np.float32)
        onehot[c] = 1.0
        gpk[0:FD, SEL0 : SEL0 + BS] = onehot[None, :]
        # xt[sb*128+p, ci*512+s] = x[c][sb*512+s, ci*128+p]
        xt = (
            x[c]
            .reshape(NSB, SBW, NC_I, P)
            .transpose(0, 3, 2, 1)
            .reshape(NSB * P, NC_I * SBW)
        )
        in_maps.append(dict(
            gpk=np.ascontiguousarray(gpk),
            wap=wap, wbp=wbp,
            xt=np.ascontiguousarray(xt).astype(BF16),
        ))
    return in_maps


def unscramble_y(y_dev):
    """y_dev [1024, 8192] bf16 -> y [2048, 4096] f32.

    y_dev[(sb*2+th)*128 + p, j*4096 + o] = y[(sb*4 + th*2 + j)*128 + p, o]
    """
    y = np.asarray(y_dev).reshape(NSB, 2, P, 2, OUT).transpose(0, 1, 3, 2, 4)
    return np.ascontiguousarray(y.reshape(SEQ, OUT)).astype(np.float32)


def get_compiled():
    global _COMPILED
    if _COMPILED is None:
        _COMPILED = build_program()
    return _COMPILED


def run(inputs, trace=False):
    from concourse.bass_utils import run_bass_kernel_spmd

    nc = get_compiled()
    in_maps = host_prep(inputs)
    res = run_bass_kernel_spmd(nc, in_maps, list(range(BS)), trace=trace)
    out = np.stack([unscramble_y(res.results[c]["y"]) for c in range(BS)], axis=0)
    return out, res


def kernel(**inputs) -> np.ndarray:
    out, _ = run(inputs, trace=False)
    return out
